# revision 15
# baseline (speedup 1.0000x reference)
"""Trainium2 Bass kernel for EnhancedMultiHeadAttention (B=2, S=2048, DM=1024, H=16).

Sharding: 8 NeuronCores = 2 batches x 4 query-row blocks of 512 rows. Each
core computes K/V for its whole batch (4x redundant; no collectives), plus
attention, output projection, gate and layernorm for its own 512 query rows.
The host concatenates the 8 output shards.

v2: mixed fp8 precision to pull the PE stream (~307us busy in the fp32r
baseline) under the ScalarE GELU stream (~137us, fixed: 16.8M exact-erf
elements/core at 1 elem/cycle/lane @1.2GHz). fp8e4m3 + DoubleRow perf mode
runs 2 k-tiles per instruction at 0.5 cyc/row = 4x fp32r throughput. Config
(validated in numpy at rel err ~1.0e-2 vs the 2e-2 gate):
  - K-proj, V-proj: fp8 DR (x and Wk/Wv host-quantized to e4m3)
  - Q-proj: f32r from a separate f32 copy of the core's own 512 x-columns
    (kills the x-quantization error through q; scores see exact q)
  - scores: f32r (same speed as fp8 without DR since contraction is 64)
  - GELU out (att): fp8; attn@v: fp8 DR over t-tile pairs (4x)
  - out-projs (gate + residual paths) and gate matmul: fp8 DR; sigmoid exact
head_w = softmax(attention_weights) is NOT folded into Wv (1/16-scaled Wv
would land in e4m3's subnormal range); it is applied per-partition at the
ctx PSUM->SBUF copy instead. PE total ~309K cycles (~129us) under the Act
stream; epilogue weight/bias/xr DMAs are prefetched behind attention.

Walrus ISA constraint (probed): DoubleRow rejects tile_position with a
column offset, so the two heads of a pair cannot be col-packed into one
PSUM bank. attn@v instead runs per-head (M=64, base partition 0) into two
separate PSUM tiles, and ctx lives in a per-head [64, H, SQ] layout; the
out-projections contract it in 64-deep DR chunks (allowed).
"""
import math
import os
import sys

import numpy as np

for _p in ("/opt/trn_rl_repo", "/opt/pypackages"):
    if _p not in sys.path:
        sys.path.append(_p)

import ml_dtypes

import concourse.bass as bass
import concourse.mybir as mybir
import concourse.tile as tile
from concourse import bacc
from concourse.bass_utils import run_bass_kernel_spmd

F32R = mybir.dt.float32r
F32 = mybir.dt.float32
BF16 = mybir.dt.bfloat16
F8 = mybir.dt.float8e4
AF = mybir.ActivationFunctionType
ALU = mybir.AluOpType
DR = mybir.MatmulPerfMode.DoubleRow

B, S, DM, H = 2, 2048, 1024, 16
HD = DM // H                  # 64
SQ = 512                      # query rows per core
NP = 128                      # partitions
KC = DM // NP                 # 8 contraction chunks
NT = S // NP                  # 16 key/value tiles
NPAIR = H // 2                # 8 head pairs
NST = SQ // NP                # 4 row tiles in row-layout phases
N512 = 512
NQ = 512                      # v-projection column half width
SCALE = 1.0 / math.sqrt(HD)
EPS = 1e-5

_CACHE = {}
_TRACE = [False]
_LAST_RESULT = [None]


def _bcast(ap_1d, p=NP):
    return bass.AP(tensor=ap_1d.tensor, offset=ap_1d.offset,
                   ap=[[0, p]] + list(ap_1d.ap))


def _build():
    nc = bacc.Bacc("TRN2", target_bir_lowering=False, debug=False)

    xT8_d = nc.dram_tensor("xT8", [DM, S], F8, kind="ExternalInput").ap()
    xTq_d = nc.dram_tensor("xTq", [DM, SQ], F32R, kind="ExternalInput").ap()
    xr_d = nc.dram_tensor("xr", [SQ, DM], F32, kind="ExternalInput").ap()
    wk8_d = nc.dram_tensor("wk8", [DM, DM], F8, kind="ExternalInput").ap()
    wv8_d = nc.dram_tensor("wv8", [DM, DM], F8, kind="ExternalInput").ap()
    wqT_d = nc.dram_tensor("wqT", [DM, DM], F32R, kind="ExternalInput").ap()
    wo8_d = nc.dram_tensor("wo8", [DM, DM], F8, kind="ExternalInput").ap()
    wg8_d = nc.dram_tensor("wg8", [DM, DM], F8, kind="ExternalInput").ap()
    bq_d = nc.dram_tensor("bq", [DM], F32, kind="ExternalInput").ap()
    bk_d = nc.dram_tensor("bk", [DM], F32, kind="ExternalInput").ap()
    bv_d = nc.dram_tensor("bv", [DM], F32, kind="ExternalInput").ap()
    bo_d = nc.dram_tensor("bo", [DM], F32, kind="ExternalInput").ap()
    bg_d = nc.dram_tensor("bg", [DM], F32, kind="ExternalInput").ap()
    hw_d = nc.dram_tensor("hwp", [H * 64], F32, kind="ExternalInput").ap()
    gam_d = nc.dram_tensor("gam", [DM], F32, kind="ExternalInput").ap()
    bet_d = nc.dram_tensor("bet", [DM], F32, kind="ExternalInput").ap()
    y_d = nc.dram_tensor("y", [SQ, DM], F32, kind="ExternalOutput").ap()

    x8_v = xT8_d.rearrange("(c p) s -> p c s", p=NP)
    xq_v = xTq_d.rearrange("(c p) s -> p c s", p=NP)
    wk_v = wk8_d.rearrange("(c p) d -> p c d", p=NP)
    wv_v = wv8_d.rearrange("(c p) d -> p c d", p=NP)
    wq_v = wqT_d.rearrange("(c p) d -> p c d", p=NP)
    wo8_v = wo8_d.rearrange("(h p) d -> p h d", p=64)
    wg8_v = wg8_d.rearrange("(c p) d -> p c d", p=NP)

    with tile.TileContext(nc) as tc:
        with tc.tile_pool(name="pers", bufs=1) as pers, \
             tc.tile_pool(name="acc", bufs=1) as acc, \
             tc.tile_pool(name="epi", bufs=1) as epi:
            bq_sb = pers.tile([NP, KC], F32)
            bk_sb = pers.tile([NP, KC], F32)
            bo_sb = pers.tile([NP, KC], F32)
            nc.sync.dma_start(out=bq_sb, in_=bq_d.rearrange("(c p) -> p c", p=NP))
            nc.sync.dma_start(out=bk_sb, in_=bk_d.rearrange("(c p) -> p c", p=NP))
            nc.sync.dma_start(out=bo_sb, in_=bo_d.rearrange("(c p) -> p c", p=NP))
            bv_bc = pers.tile([NP, DM], F32)
            nc.sync.dma_start(out=bv_bc, in_=_bcast(bv_d))
            hw_sb = pers.tile([64, H], F32)
            nc.sync.dma_start(out=hw_sb, in_=hw_d.rearrange("(c p) -> p c", p=64))
            eps_sb = pers.tile([NP, 1], F32)
            nc.vector.memset(eps_sb, EPS)

            ctx8_sb = acc.tile([64, H, SQ], F8)

            # epilogue tiles allocated up-front so their DMAs overlap the
            # attention window
            wo8_sb = epi.tile([64, H, DM], F8)
            wg8_sb = epi.tile([NP, KC, DM], F8)
            bo_bc = epi.tile([NP, DM], F32)
            bg_bc = epi.tile([NP, DM], F32)
            gam_bc = epi.tile([NP, DM], F32)
            bet_bc = epi.tile([NP, DM], F32)
            xr_sb = epi.tile([NP, NST, DM], F32)
            outT8_sb = epi.tile([NP, KC, SQ], F8)

            def epilogue_dmas():
                for dt in range(KC):
                    nc.sync.dma_start(out=wo8_sb[:, :, dt * NP:(dt + 1) * NP],
                                      in_=wo8_v[:, :, dt * NP:(dt + 1) * NP])
                    nc.sync.dma_start(out=wg8_sb[:, :, dt * NP:(dt + 1) * NP],
                                      in_=wg8_v[:, :, dt * NP:(dt + 1) * NP])
                nc.sync.dma_start(out=bo_bc, in_=_bcast(bo_d))
                nc.sync.dma_start(out=bg_bc, in_=_bcast(bg_d))
                nc.sync.dma_start(out=gam_bc, in_=_bcast(gam_d))
                nc.sync.dma_start(out=bet_bc, in_=_bcast(bet_d))
                nc.sync.dma_start(out=xr_sb,
                                  in_=xr_d.rearrange("(n p) d -> p n d", p=NP))

            with tc.tile_pool(name="xres", bufs=1) as xres, \
                 tc.tile_pool(name="qres", bufs=1) as qres, \
                 tc.tile_pool(name="wvp", bufs=1) as wvp, \
                 tc.tile_pool(name="wsl", bufs=2) as wsl, \
                 tc.tile_pool(name="kpp", bufs=2) as kpp, \
                 tc.tile_pool(name="vqp", bufs=2) as vqp, \
                 tc.tile_pool(name="attp", bufs=3) as attp, \
                 tc.tile_pool(name="pp", bufs=2, space="PSUM") as pp, \
                 tc.tile_pool(name="scop", bufs=2, space="PSUM") as scop, \
                 tc.tile_pool(name="cxp", bufs=1, space="PSUM") as cxp:
                x8_sb = xres.tile([NP, KC, S], F8)
                xq_sb = xres.tile([NP, KC, SQ], F32R)
                qT_sb = qres.tile([NP, KC, SQ], F32R)

                v_q = [None] * 2
                wv_sbs = [None] * 2

                def v_open(q):
                    wv_sb = wvp.tile([NP, KC, NQ], F8, tag="wv", name="wv_sb")
                    nc.gpsimd.dma_start(out=wv_sb,
                                        in_=wv_v[:, :, q * NQ:(q + 1) * NQ])
                    wv_sbs[q] = wv_sb
                    vq = vqp.tile([NP, NT, NQ], F8, tag="vq", name="vq")
                    v_q[q] = vq

                def v_tt(q, tt):
                    ps_t = pp.tile([NP, NQ], F32, tag="pj", name="ps_t")
                    for kc in range(0, KC, 2):
                        nc.tensor.matmul(
                            ps_t,
                            x8_sb[:, kc:kc + 2, tt * NP:(tt + 1) * NP],
                            wv_sbs[q][:, kc:kc + 2, :],
                            start=(kc == 0), stop=(kc == KC - 2),
                            perf_mode=DR)
                    nc.vector.tensor_add(
                        v_q[q][:, tt, :], ps_t, bv_bc[:, q * NQ:(q + 1) * NQ])

                kpairs = [None] * NPAIR
                wk_tiles = {}

                def wk_dma(p):
                    wk_sl = wsl.tile([NP, KC, NP], F8, tag="wk", name="wk_sl")
                    nc.sync.dma_start(out=wk_sl,
                                      in_=wk_v[:, :, p * NP:(p + 1) * NP])
                    wk_tiles[p] = wk_sl

                def prepare(p):
                    # kpair[d, t] = sum_k Wk[d, k] x[t, k] + bk[d], d in pair rows
                    if p not in wk_tiles:
                        wk_dma(p)
                    wk_sl = wk_tiles[p]
                    kpair = kpp.tile([NP, S], F32R, tag="kp", name="kpair")
                    for ts in range(S // N512):
                        ps_t = pp.tile([NP, N512], F32, tag="pj", name="ps_t")
                        for kc in range(0, KC, 2):
                            nc.tensor.matmul(
                                ps_t,
                                wk_sl[:, kc:kc + 2, :],
                                x8_sb[:, kc:kc + 2, ts * N512:(ts + 1) * N512],
                                start=(kc == 0), stop=(kc == KC - 2),
                                perf_mode=DR)
                        nc.vector.tensor_scalar_add(
                            kpair[:, ts * N512:(ts + 1) * N512], ps_t,
                            bk_sb[:, p:p + 1])
                    kpairs[p] = kpair

                def attn(p, pre_t=None):
                    kpair = kpairs[p]
                    vq = v_q[p // 4]
                    c0 = (p % 4) * NP
                    ctxa_ps = cxp.tile([64, SQ], F32, tag="cxa", name="ctxa_ps")
                    ctxb_ps = cxp.tile([64, SQ], F32, tag="cxb", name="ctxb_ps")
                    for tp in range(NT // 2):
                        att2 = attp.tile([NP, 2, 2 * SQ], F8, tag="at",
                                         name="att2")
                        for j in range(2):
                            t = 2 * tp + j
                            if pre_t is not None:
                                pre_t(t)
                            sco = scop.tile([NP, 2 * SQ], F32, tag="sc",
                                            name="sco")
                            nc.tensor.matmul(sco[:, 0:SQ],
                                             kpair[0:64, t * NP:(t + 1) * NP],
                                             qT_sb[0:64, p, :],
                                             start=True, stop=True,
                                             tile_position=(0, 0))
                            nc.tensor.matmul(sco[:, SQ:2 * SQ],
                                             kpair[64:128, t * NP:(t + 1) * NP],
                                             qT_sb[64:128, p, :],
                                             start=True, stop=True,
                                             tile_position=(64, 0))
                            nc.scalar.activation(out=att2[:, j, :], in_=sco,
                                                 func=AF.Gelu, scale=SCALE)
                        nc.tensor.matmul(ctxa_ps,
                                         vq[:, 2 * tp:2 * tp + 2, c0:c0 + 64],
                                         att2[:, :, 0:SQ],
                                         start=(tp == 0), stop=(tp == NT // 2 - 1),
                                         perf_mode=DR, tile_position=(0, 0))
                        nc.tensor.matmul(ctxb_ps,
                                         vq[:, 2 * tp:2 * tp + 2, c0 + 64:c0 + NP],
                                         att2[:, :, SQ:2 * SQ],
                                         start=(tp == 0), stop=(tp == NT // 2 - 1),
                                         perf_mode=DR, tile_position=(0, 0))
                    nc.vector.tensor_scalar_mul(ctx8_sb[:, 2 * p, :], ctxa_ps,
                                                hw_sb[:, 2 * p:2 * p + 1])
                    nc.vector.tensor_scalar_mul(ctx8_sb[:, 2 * p + 1, :], ctxb_ps,
                                                hw_sb[:, 2 * p + 1:2 * p + 2])

                # opening DMA order: wk(0), x8 query-cols chunk + xq, then the
                # Q projection (wq slices land right behind), then rest of x8,
                # then the epilogue prefetch
                wk_dma(0)
                for kc in range(KC):
                    nc.sync.dma_start(
                        out=x8_sb[:, kc, 0:N512], in_=x8_v[:, kc, 0:N512])
                    nc.sync.dma_start(out=xq_sb[:, kc, :], in_=xq_v[:, kc, :])
                for dt in range(KC):
                    wq_sl = wsl.tile([NP, KC, NP], F32R, tag="wq", name="wq_sl")
                    nc.sync.dma_start(out=wq_sl,
                                      in_=wq_v[:, :, dt * NP:(dt + 1) * NP])
                    ps_q = pp.tile([NP, SQ], F32, tag="pj", name="ps_q")
                    for kc in range(KC):
                        nc.tensor.matmul(ps_q, wq_sl[:, kc, :],
                                         xq_sb[:, kc, :],
                                         start=(kc == 0), stop=(kc == KC - 1))
                    nc.vector.tensor_scalar_add(qT_sb[:, dt, :], ps_q,
                                                bq_sb[:, dt:dt + 1])
                for ts in range(1, S // N512):
                    for kc in range(KC):
                        nc.sync.dma_start(
                            out=x8_sb[:, kc, ts * N512:(ts + 1) * N512],
                            in_=x8_v[:, kc, ts * N512:(ts + 1) * N512])
                epilogue_dmas()

                prepare(0)
                v_open(0)
                attn(0, pre_t=lambda t: v_tt(0, t))
                prepare(1)
                v_open(1)
                for _tt in range(0, 4):
                    v_tt(1, _tt)
                attn(1)
                prepare(2)
                for _tt in range(4, 8):
                    v_tt(1, _tt)
                attn(2)
                prepare(3)
                for _tt in range(8, 12):
                    v_tt(1, _tt)
                attn(3)
                prepare(4)
                for _tt in range(12, NT):
                    v_tt(1, _tt)
                attn(4)
                prepare(5)
                attn(5)
                prepare(6)
                attn(6)
                prepare(7)
                attn(7)

            # ------------- out proj, gate + epilogue -----------------------
            with tc.tile_pool(name="big", bufs=1) as big, \
                 tc.tile_pool(name="pp2", bufs=4, space="PSUM") as pp2:
                # gate-path out projection, fp8 DR over 64-deep head chunks
                for dt in range(KC):
                    ps_t = pp2.tile([NP, SQ], F32, tag="po", name="ps_t")
                    for h in range(0, H, 2):
                        nc.tensor.matmul(
                            ps_t,
                            wo8_sb[:, h:h + 2, dt * NP:(dt + 1) * NP],
                            ctx8_sb[:, h:h + 2, :],
                            start=(h == 0), stop=(h == H - 2),
                            perf_mode=DR)
                    nc.vector.tensor_scalar_add(outT8_sb[:, dt, :], ps_t,
                                                bo_sb[:, dt:dt + 1])
                orow_sb = big.tile([NP, NST, DM], F32)
                gate_sb = big.tile([NP, NST, DM], F32)
                t1_sb = big.tile([NP, NST, DM], F32)
                stats = pers.tile([NP, 2, 6], F32)
                mv = pers.tile([NP, 2], F32)
                std = pers.tile([NP, 1], F32)
                rstd = pers.tile([NP, 1], F32)
                y_sb = gate_sb
                for st in range(NST):
                    for ns in range(DM // N512):
                        ps_t = pp2.tile([NP, N512], F32, tag="po", name="ps_t")
                        for dc in range(0, KC, 2):
                            nc.tensor.matmul(
                                ps_t,
                                outT8_sb[:, dc:dc + 2, st * NP:(st + 1) * NP],
                                wg8_sb[:, dc:dc + 2, ns * N512:(ns + 1) * N512],
                                start=(dc == 0), stop=(dc == KC - 2),
                                perf_mode=DR)
                        nc.vector.tensor_add(
                            gate_sb[:, st, ns * N512:(ns + 1) * N512],
                            ps_t, bg_bc[:, ns * N512:(ns + 1) * N512])
                    nc.scalar.activation(out=gate_sb[:, st, :],
                                         in_=gate_sb[:, st, :], func=AF.Sigmoid)
                    for ns in range(DM // N512):
                        ps_t = pp2.tile([NP, N512], F32, tag="po", name="ps_t")
                        for h in range(0, H, 2):
                            nc.tensor.matmul(
                                ps_t,
                                ctx8_sb[:, h:h + 2, st * NP:(st + 1) * NP],
                                wo8_sb[:, h:h + 2, ns * N512:(ns + 1) * N512],
                                start=(h == 0), stop=(h == H - 2),
                                perf_mode=DR)
                        nc.vector.tensor_add(
                            orow_sb[:, st, ns * N512:(ns + 1) * N512],
                            ps_t, bo_bc[:, ns * N512:(ns + 1) * N512])
                    # y_pre = gate*(out - x) + 2x
                    nc.vector.tensor_sub(t1_sb[:, st, :], orow_sb[:, st, :],
                                         xr_sb[:, st, :])
                    nc.vector.tensor_mul(orow_sb[:, st, :], t1_sb[:, st, :],
                                         gate_sb[:, st, :])
                    nc.vector.scalar_tensor_tensor(
                        out=t1_sb[:, st, :], in0=xr_sb[:, st, :], scalar=2.0,
                        in1=orow_sb[:, st, :], op0=ALU.mult, op1=ALU.add)
                    # layernorm over DM
                    yv = t1_sb[:, st, :].rearrange("p (g d) -> p g d", g=2)
                    for g in range(2):
                        nc.vector.bn_stats(out=stats[:, g, :], in_=yv[:, g, :])
                    nc.vector.bn_aggr(out=mv, in_=stats)
                    nc.scalar.activation(out=std, in_=mv[:, 1:2], func=AF.Sqrt,
                                         bias=eps_sb)
                    nc.vector.reciprocal(rstd, std)
                    nc.vector.tensor_scalar(
                        out=orow_sb[:, st, :], in0=t1_sb[:, st, :],
                        scalar1=mv[:, 0:1], scalar2=rstd,
                        op0=ALU.subtract, op1=ALU.mult)
                    nc.vector.tensor_mul(orow_sb[:, st, :], orow_sb[:, st, :],
                                         gam_bc)
                    nc.vector.tensor_add(y_sb[:, st, :], orow_sb[:, st, :],
                                         bet_bc)
                    nc.sync.dma_start(
                        out=y_d.rearrange("(n p) d -> p n d", p=NP)[:, st, :],
                        in_=y_sb[:, st, :])

    nc.compile()
    return nc


def kernel(x, Wq, bq, Wk, bk, Wv, bv, Wo, bo, Wg, bg, attention_weights,
           ln_gamma, ln_beta):
    x = np.asarray(x, dtype=np.float32)
    f32 = lambda a: np.ascontiguousarray(np.asarray(a, dtype=np.float32))
    f8 = lambda a: np.ascontiguousarray(
        np.asarray(a, dtype=np.float32).astype(ml_dtypes.float8_e4m3fn))
    bf16 = lambda a: np.ascontiguousarray(
        np.asarray(a, dtype=np.float32).astype(ml_dtypes.bfloat16))
    Wq, Wk, Wv, Wo, Wg = map(f32, (Wq, Wk, Wv, Wo, Wg))
    bq, bk, bv, bo, bg = map(f32, (bq, bk, bv, bo, bg))
    aw, gam, bet = map(f32, (attention_weights, ln_gamma, ln_beta))

    if "nc" not in _CACHE:
        _CACHE["nc"] = _build()
    nc = _CACHE["nc"]

    e = np.exp(aw - aw.max())
    head_w = (e / e.sum()).astype(np.float32)
    # per-head scalar replicated over 64 partitions: hwp[h*64 + p] = head_w[h]
    hwp = np.repeat(head_w, 64).astype(np.float32)

    wqT = np.ascontiguousarray(Wq.T)
    wk8 = f8(Wk.T)
    wv8 = f8(Wv.T)
    wo8 = f8(Wo.T)
    wg8 = f8(Wg.T)

    in_maps = []
    for c in range(8):
        b, blk = divmod(c, 4)
        r0 = blk * SQ
        xb = x[b]
        perm = np.r_[r0:r0 + SQ, 0:r0, r0 + SQ:S]
        in_maps.append({
            "xT8": f8(xb[perm].T),
            "xTq": np.ascontiguousarray(xb[r0:r0 + SQ].T),
            "xr": np.ascontiguousarray(xb[r0:r0 + SQ]),
            "wk8": wk8, "wv8": wv8, "wqT": wqT,
            "wo8": wo8, "wg8": wg8,
            "bq": bq, "bk": bk, "bv": bv, "bo": bo, "bg": bg,
            "hwp": hwp, "gam": gam, "bet": bet,
        })

    last_exc = None
    for _attempt in range(3):
        try:
            res = run_bass_kernel_spmd(nc, in_maps, core_ids=list(range(8)),
                                       trace=_TRACE[0])
            break
        except Exception as exc:  # flaky NRT_EXEC_UNIT errors: retry
            last_exc = exc
            import time
            time.sleep(2.0)
    else:
        raise last_exc
    _LAST_RESULT[0] = res

    y = np.empty((B, S, DM), dtype=np.float32)
    for c in range(8):
        b, blk = divmod(c, 4)
        r0 = blk * SQ
        y[b, r0:r0 + SQ] = res.results[c]["y"]
    return y


# revision 20
# speedup vs baseline: 1.0887x; 1.0887x over previous
"""Trainium2 Bass kernel for EnhancedMultiHeadAttention (B=2, S=2048, DM=1024, H=16).

Sharding: 8 NeuronCores = 2 batches x 4 query-row blocks of 512 rows. Each
core computes K/V for its whole batch (4x redundant; no collectives), plus
attention, output projection, gate and layernorm for its own 512 query rows.
The host concatenates the 8 output shards.

v2: mixed fp8 precision to pull the PE stream (~307us busy in the fp32r
baseline) under the ScalarE GELU stream (~137us, fixed: 16.8M exact-erf
elements/core at 1 elem/cycle/lane @1.2GHz). fp8e4m3 + DoubleRow perf mode
runs 2 k-tiles per instruction at 0.5 cyc/row = 4x fp32r throughput. Config
(validated in numpy at rel err ~1.0e-2 vs the 2e-2 gate):
  - K-proj, V-proj: fp8 DR (x and Wk/Wv host-quantized to e4m3)
  - Q-proj: f32r from a separate f32 copy of the core's own 512 x-columns
    (kills the x-quantization error through q; scores see exact q)
  - scores: f32r (same speed as fp8 without DR since contraction is 64)
  - GELU out (att): fp8; attn@v: fp8 DR over t-tile pairs (4x)
  - out-projs (gate + residual paths) and gate matmul: fp8 DR; sigmoid exact
head_w = softmax(attention_weights) is NOT folded into Wv (1/16-scaled Wv
would land in e4m3's subnormal range); it is applied per-partition at the
ctx PSUM->SBUF copy instead. PE total ~309K cycles (~129us) under the Act
stream; epilogue weight/bias/xr DMAs are prefetched behind attention.

Walrus ISA constraint (probed): DoubleRow rejects tile_position with a
column offset, so the two heads of a pair cannot be col-packed into one
PSUM bank. attn@v instead runs per-head (M=64, base partition 0) into two
separate PSUM tiles, and ctx lives in a per-head [64, H, SQ] layout; the
out-projections contract it in 64-deep DR chunks (allowed).
"""
import math
import os
import sys

import numpy as np

for _p in ("/opt/trn_rl_repo", "/opt/pypackages"):
    if _p not in sys.path:
        sys.path.append(_p)

import ml_dtypes

import concourse.bass as bass
import concourse.mybir as mybir
import concourse.tile as tile
from concourse import bacc
from concourse.bass_utils import run_bass_kernel_spmd

F32R = mybir.dt.float32r
F32 = mybir.dt.float32
BF16 = mybir.dt.bfloat16
F8 = mybir.dt.float8e4
AF = mybir.ActivationFunctionType
ALU = mybir.AluOpType
DR = mybir.MatmulPerfMode.DoubleRow

B, S, DM, H = 2, 2048, 1024, 16
HD = DM // H                  # 64
SQ = 512                      # query rows per core
NP = 128                      # partitions
KC = DM // NP                 # 8 contraction chunks
NT = S // NP                  # 16 key/value tiles
NPAIR = H // 2                # 8 head pairs
NST = SQ // NP                # 4 row tiles in row-layout phases
N512 = 512
NQ = 512                      # v-projection column half width
SCALE = 1.0 / math.sqrt(HD)
EPS = 1e-5

_CACHE = {}
_TRACE = [False]
_LAST_RESULT = [None]


def _bcast(ap_1d, p=NP):
    return bass.AP(tensor=ap_1d.tensor, offset=ap_1d.offset,
                   ap=[[0, p]] + list(ap_1d.ap))


def _build():
    nc = bacc.Bacc("TRN2", target_bir_lowering=False, debug=False)

    xT8_d = nc.dram_tensor("xT8", [DM, S], F8, kind="ExternalInput").ap()
    xTq_d = nc.dram_tensor("xTq", [DM, SQ], F32R, kind="ExternalInput").ap()
    xr_d = nc.dram_tensor("xr", [SQ, DM], F32, kind="ExternalInput").ap()
    wk8_d = nc.dram_tensor("wk8", [DM, DM], F8, kind="ExternalInput").ap()
    wv8_d = nc.dram_tensor("wv8", [DM, DM], F8, kind="ExternalInput").ap()
    wqT_d = nc.dram_tensor("wqT", [DM, DM], F32R, kind="ExternalInput").ap()
    wo8_d = nc.dram_tensor("wo8", [DM, DM], F8, kind="ExternalInput").ap()
    wg8_d = nc.dram_tensor("wg8", [DM, DM], F8, kind="ExternalInput").ap()
    bq_d = nc.dram_tensor("bq", [DM], F32, kind="ExternalInput").ap()
    bk_d = nc.dram_tensor("bk", [DM], F32, kind="ExternalInput").ap()
    bv_d = nc.dram_tensor("bv", [DM], F32, kind="ExternalInput").ap()
    bo_d = nc.dram_tensor("bo", [DM], F32, kind="ExternalInput").ap()
    bg_d = nc.dram_tensor("bg", [DM], F32, kind="ExternalInput").ap()
    hw_d = nc.dram_tensor("hwp", [H * 64], F32, kind="ExternalInput").ap()
    gam_d = nc.dram_tensor("gam", [DM], F32, kind="ExternalInput").ap()
    bet_d = nc.dram_tensor("bet", [DM], F32, kind="ExternalInput").ap()
    y_d = nc.dram_tensor("y", [SQ, DM], F32, kind="ExternalOutput").ap()

    x8_v = xT8_d.rearrange("(c p) s -> p c s", p=NP)
    xq_v = xTq_d.rearrange("(c p) s -> p c s", p=NP)
    wk_v = wk8_d.rearrange("(c p) d -> p c d", p=NP)
    wv_v = wv8_d.rearrange("(c p) d -> p c d", p=NP)
    wq_v = wqT_d.rearrange("(c p) d -> p c d", p=NP)
    wo8_v = wo8_d.rearrange("(h p) d -> p h d", p=64)
    wg8_v = wg8_d.rearrange("(c p) d -> p c d", p=NP)

    with tile.TileContext(nc) as tc:
        with tc.tile_pool(name="pers", bufs=1) as pers, \
             tc.tile_pool(name="acc", bufs=1) as acc, \
             tc.tile_pool(name="epi", bufs=1) as epi:
            bq_sb = pers.tile([NP, KC], F32)
            bk_sb = pers.tile([NP, KC], F32)
            bo_sb = pers.tile([NP, KC], F32)
            nc.sync.dma_start(out=bq_sb, in_=bq_d.rearrange("(c p) -> p c", p=NP))
            nc.sync.dma_start(out=bk_sb, in_=bk_d.rearrange("(c p) -> p c", p=NP))
            nc.sync.dma_start(out=bo_sb, in_=bo_d.rearrange("(c p) -> p c", p=NP))
            bv_bc = pers.tile([NP, DM], F32)
            nc.sync.dma_start(out=bv_bc, in_=_bcast(bv_d))
            hw_sb = pers.tile([64, H], F32)
            nc.sync.dma_start(out=hw_sb, in_=hw_d.rearrange("(c p) -> p c", p=64))
            eps_sb = pers.tile([NP, 1], F32)
            nc.vector.memset(eps_sb, EPS)

            ctx8_sb = acc.tile([64, H, SQ], F8)

            # epilogue tiles allocated up-front so their DMAs overlap the
            # attention window
            wo8_sb = epi.tile([64, H, DM], F8)
            wg8_sb = epi.tile([NP, KC, DM], F8)
            bo_bc = epi.tile([NP, DM], F32)
            bg_bc = epi.tile([NP, DM], F32)
            gam_bc = epi.tile([NP, DM], F32)
            bet_bc = epi.tile([NP, DM], F32)
            xr_sb = epi.tile([NP, NST, DM], F32)
            outT8_sb = epi.tile([NP, KC, SQ], F8)

            def epilogue_dmas():
                for dt in range(KC):
                    nc.sync.dma_start(out=wo8_sb[:, :, dt * NP:(dt + 1) * NP],
                                      in_=wo8_v[:, :, dt * NP:(dt + 1) * NP])
                    nc.sync.dma_start(out=wg8_sb[:, :, dt * NP:(dt + 1) * NP],
                                      in_=wg8_v[:, :, dt * NP:(dt + 1) * NP])
                nc.sync.dma_start(out=bo_bc, in_=_bcast(bo_d))
                nc.sync.dma_start(out=bg_bc, in_=_bcast(bg_d))
                nc.sync.dma_start(out=gam_bc, in_=_bcast(gam_d))
                nc.sync.dma_start(out=bet_bc, in_=_bcast(bet_d))
                nc.sync.dma_start(out=xr_sb,
                                  in_=xr_d.rearrange("(n p) d -> p n d", p=NP))

            with tc.tile_pool(name="xres", bufs=1) as xres, \
                 tc.tile_pool(name="qres", bufs=1) as qres, \
                 tc.tile_pool(name="wvp", bufs=1) as wvp, \
                 tc.tile_pool(name="wsl", bufs=2) as wsl, \
                 tc.tile_pool(name="kpp", bufs=2) as kpp, \
                 tc.tile_pool(name="vqp", bufs=2) as vqp, \
                 tc.tile_pool(name="attp", bufs=3) as attp, \
                 tc.tile_pool(name="pp", bufs=2, space="PSUM") as pp, \
                 tc.tile_pool(name="scop", bufs=2, space="PSUM") as scop, \
                 tc.tile_pool(name="cxp", bufs=1, space="PSUM") as cxp:
                x8_sb = xres.tile([NP, KC, S], F8)
                xq_sb = xres.tile([NP, KC, SQ], F32R)
                qT_sb = qres.tile([NP, KC, SQ], F32R)

                v_q = [None] * 2
                wv_sbs = [None] * 2

                def v_open(q):
                    wv_sb = wvp.tile([NP, KC, NQ], F8, tag="wv", name="wv_sb")
                    nc.gpsimd.dma_start(out=wv_sb,
                                        in_=wv_v[:, :, q * NQ:(q + 1) * NQ])
                    wv_sbs[q] = wv_sb
                    vq = vqp.tile([NP, NT, NQ], F8, tag="vq", name="vq")
                    v_q[q] = vq

                def v_tt(q, tt):
                    # fp8 without DoubleRow on purpose: the extra occupancy
                    # keeps the PE HAM clock pinned at max pstate (a sparse
                    # PE stream measurably downclocks to 1.2/0.65 GHz and
                    # stretches the score matmuls feeding the GELU stream)
                    ps_t = pp.tile([NP, NQ], F32, tag="pj", name="ps_t")
                    for kc in range(KC):
                        nc.tensor.matmul(
                            ps_t,
                            x8_sb[:, kc, tt * NP:(tt + 1) * NP],
                            wv_sbs[q][:, kc, :],
                            start=(kc == 0), stop=(kc == KC - 1))
                    nc.vector.tensor_add(
                        v_q[q][:, tt, :], ps_t, bv_bc[:, q * NQ:(q + 1) * NQ])

                kpairs = [None] * NPAIR
                wk_tiles = {}

                def wk_dma(p):
                    wk_sl = wsl.tile([NP, KC, NP], F8, tag="wk", name="wk_sl")
                    nc.sync.dma_start(out=wk_sl,
                                      in_=wk_v[:, :, p * NP:(p + 1) * NP])
                    wk_tiles[p] = wk_sl

                def prepare_ts(p, ts):
                    # kpair[d, t] = sum_k Wk[d, k] x[t, k] + bk[d], d in pair rows
                    wk_sl = wk_tiles[p]
                    kpair = kpairs[p]
                    ps_t = pp.tile([NP, N512], F32, tag="pj", name="ps_t")
                    for kc in range(0, KC, 2):
                        nc.tensor.matmul(
                            ps_t,
                            wk_sl[:, kc:kc + 2, :],
                            x8_sb[:, kc:kc + 2, ts * N512:(ts + 1) * N512],
                            start=(kc == 0), stop=(kc == KC - 2),
                            perf_mode=DR)
                    nc.vector.tensor_scalar_add(
                        kpair[:, ts * N512:(ts + 1) * N512], ps_t,
                        bk_sb[:, p:p + 1])

                def prepare(p):
                    if p not in wk_tiles:
                        wk_dma(p)
                    kpairs[p] = kpp.tile([NP, S], F32R, tag="kp", name="kpair")
                    for ts in range(S // N512):
                        prepare_ts(p, ts)

                def attn(p, pre_t=None):
                    kpair = kpairs[p]
                    vq = v_q[p // 4]
                    c0 = (p % 4) * NP
                    ctxa_ps = cxp.tile([64, SQ], F32, tag="cxa", name="ctxa_ps")
                    ctxb_ps = cxp.tile([64, SQ], F32, tag="cxb", name="ctxb_ps")
                    for tp in range(NT // 2):
                        att2 = attp.tile([NP, 2, 2 * SQ], F8, tag="at",
                                         name="att2")
                        for j in range(2):
                            t = 2 * tp + j
                            if pre_t is not None:
                                pre_t(t)
                            sco = scop.tile([NP, 2 * SQ], F32, tag="sc",
                                            name="sco")
                            nc.tensor.matmul(sco[:, 0:SQ],
                                             kpair[0:64, t * NP:(t + 1) * NP],
                                             qT_sb[0:64, p, :],
                                             start=True, stop=True,
                                             tile_position=(0, 0))
                            nc.tensor.matmul(sco[:, SQ:2 * SQ],
                                             kpair[64:128, t * NP:(t + 1) * NP],
                                             qT_sb[64:128, p, :],
                                             start=True, stop=True,
                                             tile_position=(64, 0))
                            nc.scalar.activation(out=att2[:, j, :], in_=sco,
                                                 func=AF.Gelu, scale=SCALE)
                        nc.tensor.matmul(ctxa_ps,
                                         vq[:, 2 * tp:2 * tp + 2, c0:c0 + 64],
                                         att2[:, :, 0:SQ],
                                         start=(tp == 0), stop=(tp == NT // 2 - 1),
                                         perf_mode=DR, tile_position=(0, 0))
                        nc.tensor.matmul(ctxb_ps,
                                         vq[:, 2 * tp:2 * tp + 2, c0 + 64:c0 + NP],
                                         att2[:, :, SQ:2 * SQ],
                                         start=(tp == 0), stop=(tp == NT // 2 - 1),
                                         perf_mode=DR, tile_position=(0, 0))
                    nc.vector.tensor_scalar_mul(ctx8_sb[:, 2 * p, :], ctxa_ps,
                                                hw_sb[:, 2 * p:2 * p + 1])
                    nc.vector.tensor_scalar_mul(ctx8_sb[:, 2 * p + 1, :], ctxb_ps,
                                                hw_sb[:, 2 * p + 1:2 * p + 2])

                def q_proj(dt):
                    wq_sl = wsl.tile([NP, KC, NP], F32R, tag="wq", name="wq_sl")
                    nc.sync.dma_start(out=wq_sl,
                                      in_=wq_v[:, :, dt * NP:(dt + 1) * NP])
                    ps_q = pp.tile([NP, SQ], F32, tag="pj", name="ps_q")
                    for kc in range(KC):
                        nc.tensor.matmul(ps_q, wq_sl[:, kc, :],
                                         xq_sb[:, kc, :],
                                         start=(kc == 0), stop=(kc == KC - 1))
                    nc.vector.tensor_scalar_add(qT_sb[:, dt, :], ps_q,
                                                bq_sb[:, dt:dt + 1])

                # opening: wk(0) + the first x8 chunk land first so the K
                # projection starts ~2us in; xq rides the gpsimd DMA queue in
                # parallel. qT dt blocks map 1:1 to pairs, so only dt=0 is
                # projected up front — dt 1-7 interleave into attn(0)'s
                # t-loop, putting the first GELU ~25us earlier.
                wk_dma(0)
                for kc in range(KC):
                    nc.sync.dma_start(
                        out=x8_sb[:, kc, 0:N512], in_=x8_v[:, kc, 0:N512])
                nc.gpsimd.dma_start(out=xq_sb, in_=xq_v)
                kpairs[0] = kpp.tile([NP, S], F32R, tag="kp", name="kpair")
                prepare_ts(0, 0)
                q_proj(0)
                for ts in range(1, S // N512):
                    for kc in range(KC):
                        nc.sync.dma_start(
                            out=x8_sb[:, kc, ts * N512:(ts + 1) * N512],
                            in_=x8_v[:, kc, ts * N512:(ts + 1) * N512])
                    prepare_ts(0, ts)
                epilogue_dmas()

                v_open(0)

                def pre0(t):
                    v_tt(0, t)
                    if 1 <= t <= 7:
                        q_proj(t)
                attn(0, pre_t=pre0)
                prepare(1)
                v_open(1)
                for _tt in range(0, 4):
                    v_tt(1, _tt)
                attn(1)
                prepare(2)
                for _tt in range(4, 8):
                    v_tt(1, _tt)
                attn(2)
                prepare(3)
                for _tt in range(8, 12):
                    v_tt(1, _tt)
                attn(3)
                prepare(4)
                for _tt in range(12, NT):
                    v_tt(1, _tt)
                attn(4)
                prepare(5)
                attn(5)
                prepare(6)
                attn(6)
                prepare(7)
                attn(7)

            # ------------- out proj, gate + epilogue -----------------------
            with tc.tile_pool(name="big", bufs=1) as big, \
                 tc.tile_pool(name="pp2", bufs=4, space="PSUM") as pp2:
                # gate-path out projection, fp8 DR over 64-deep head chunks
                for dt in range(KC):
                    ps_t = pp2.tile([NP, SQ], F32, tag="po", name="ps_t")
                    for h in range(0, H, 2):
                        nc.tensor.matmul(
                            ps_t,
                            wo8_sb[:, h:h + 2, dt * NP:(dt + 1) * NP],
                            ctx8_sb[:, h:h + 2, :],
                            start=(h == 0), stop=(h == H - 2),
                            perf_mode=DR)
                    nc.vector.tensor_scalar_add(outT8_sb[:, dt, :], ps_t,
                                                bo_sb[:, dt:dt + 1])
                orow_sb = big.tile([NP, NST, DM], F32)
                gate_sb = big.tile([NP, NST, DM], F32)
                t1_sb = big.tile([NP, NST, DM], F32)
                stats = pers.tile([NP, 2, 6], F32)
                mv = pers.tile([NP, 2], F32)
                std = pers.tile([NP, 1], F32)
                rstd = pers.tile([NP, 1], F32)
                y_sb = gate_sb
                # all gate matmuls + sigmoids first, then the LN chain: keeps
                # the Act engine on one function table at a time (the
                # sigmoid/sqrt alternation was costing ~6us of table reloads)
                for st in range(NST):
                    for ns in range(DM // N512):
                        ps_t = pp2.tile([NP, N512], F32, tag="po", name="ps_t")
                        for dc in range(0, KC, 2):
                            nc.tensor.matmul(
                                ps_t,
                                outT8_sb[:, dc:dc + 2, st * NP:(st + 1) * NP],
                                wg8_sb[:, dc:dc + 2, ns * N512:(ns + 1) * N512],
                                start=(dc == 0), stop=(dc == KC - 2),
                                perf_mode=DR)
                        nc.vector.tensor_add(
                            gate_sb[:, st, ns * N512:(ns + 1) * N512],
                            ps_t, bg_bc[:, ns * N512:(ns + 1) * N512])
                    nc.scalar.activation(out=gate_sb[:, st, :],
                                         in_=gate_sb[:, st, :], func=AF.Sigmoid)
                for st in range(NST):
                    for ns in range(DM // N512):
                        ps_t = pp2.tile([NP, N512], F32, tag="po", name="ps_t")
                        for h in range(0, H, 2):
                            nc.tensor.matmul(
                                ps_t,
                                ctx8_sb[:, h:h + 2, st * NP:(st + 1) * NP],
                                wo8_sb[:, h:h + 2, ns * N512:(ns + 1) * N512],
                                start=(h == 0), stop=(h == H - 2),
                                perf_mode=DR)
                        nc.vector.tensor_add(
                            orow_sb[:, st, ns * N512:(ns + 1) * N512],
                            ps_t, bo_bc[:, ns * N512:(ns + 1) * N512])
                    # y_pre = gate*(out - x) + 2x
                    nc.vector.tensor_sub(t1_sb[:, st, :], orow_sb[:, st, :],
                                         xr_sb[:, st, :])
                    nc.vector.tensor_mul(orow_sb[:, st, :], t1_sb[:, st, :],
                                         gate_sb[:, st, :])
                    nc.vector.scalar_tensor_tensor(
                        out=t1_sb[:, st, :], in0=xr_sb[:, st, :], scalar=2.0,
                        in1=orow_sb[:, st, :], op0=ALU.mult, op1=ALU.add)
                    # layernorm over DM
                    yv = t1_sb[:, st, :].rearrange("p (g d) -> p g d", g=2)
                    for g in range(2):
                        nc.vector.bn_stats(out=stats[:, g, :], in_=yv[:, g, :])
                    nc.vector.bn_aggr(out=mv, in_=stats)
                    nc.scalar.activation(out=std, in_=mv[:, 1:2], func=AF.Sqrt,
                                         bias=eps_sb)
                    nc.vector.reciprocal(rstd, std)
                    nc.vector.tensor_scalar(
                        out=orow_sb[:, st, :], in0=t1_sb[:, st, :],
                        scalar1=mv[:, 0:1], scalar2=rstd,
                        op0=ALU.subtract, op1=ALU.mult)
                    nc.vector.tensor_mul(orow_sb[:, st, :], orow_sb[:, st, :],
                                         gam_bc)
                    nc.vector.tensor_add(y_sb[:, st, :], orow_sb[:, st, :],
                                         bet_bc)
                    nc.sync.dma_start(
                        out=y_d.rearrange("(n p) d -> p n d", p=NP)[:, st, :],
                        in_=y_sb[:, st, :])

    nc.compile()
    return nc


def kernel(x, Wq, bq, Wk, bk, Wv, bv, Wo, bo, Wg, bg, attention_weights,
           ln_gamma, ln_beta):
    x = np.asarray(x, dtype=np.float32)
    f32 = lambda a: np.ascontiguousarray(np.asarray(a, dtype=np.float32))
    f8 = lambda a: np.ascontiguousarray(
        np.asarray(a, dtype=np.float32).astype(ml_dtypes.float8_e4m3fn))
    bf16 = lambda a: np.ascontiguousarray(
        np.asarray(a, dtype=np.float32).astype(ml_dtypes.bfloat16))
    Wq, Wk, Wv, Wo, Wg = map(f32, (Wq, Wk, Wv, Wo, Wg))
    bq, bk, bv, bo, bg = map(f32, (bq, bk, bv, bo, bg))
    aw, gam, bet = map(f32, (attention_weights, ln_gamma, ln_beta))

    if "nc" not in _CACHE:
        _CACHE["nc"] = _build()
    nc = _CACHE["nc"]

    e = np.exp(aw - aw.max())
    head_w = (e / e.sum()).astype(np.float32)
    # per-head scalar replicated over 64 partitions: hwp[h*64 + p] = head_w[h]
    hwp = np.repeat(head_w, 64).astype(np.float32)

    wqT = np.ascontiguousarray(Wq.T)
    wk8 = f8(Wk.T)
    wv8 = f8(Wv.T)
    wo8 = f8(Wo.T)
    wg8 = f8(Wg.T)

    in_maps = []
    for c in range(8):
        b, blk = divmod(c, 4)
        r0 = blk * SQ
        xb = x[b]
        perm = np.r_[r0:r0 + SQ, 0:r0, r0 + SQ:S]
        in_maps.append({
            "xT8": f8(xb[perm].T),
            "xTq": np.ascontiguousarray(xb[r0:r0 + SQ].T),
            "xr": np.ascontiguousarray(xb[r0:r0 + SQ]),
            "wk8": wk8, "wv8": wv8, "wqT": wqT,
            "wo8": wo8, "wg8": wg8,
            "bq": bq, "bk": bk, "bv": bv, "bo": bo, "bg": bg,
            "hwp": hwp, "gam": gam, "bet": bet,
        })

    last_exc = None
    for _attempt in range(3):
        try:
            res = run_bass_kernel_spmd(nc, in_maps, core_ids=list(range(8)),
                                       trace=_TRACE[0])
            break
        except Exception as exc:  # flaky NRT_EXEC_UNIT errors: retry
            last_exc = exc
            import time
            time.sleep(2.0)
    else:
        raise last_exc
    _LAST_RESULT[0] = res

    y = np.empty((B, S, DM), dtype=np.float32)
    for c in range(8):
        b, blk = divmod(c, 4)
        r0 = blk * SQ
        y[b, r0:r0 + SQ] = res.results[c]["y"]
    return y
